# revision 36
# baseline (speedup 1.0000x reference)
"""Bahdanau-attention scores kernel for Trainium2, 8-core data-parallel.

Computes softmax_s( v . tanh(W_h @ h[b] + W_e @ enc[s,b] + bias) ) for
B=32, S=2048, Dd=512, De2=1024, sharded 4 batches per NeuronCore.

Per-core device layout (host pre-shards / pre-tiles into per-partition
form so every DMA is 128 long contiguous runs; r = b_local*2048 + s):
  encB      [128, 8*8*1024] fp16 encB[p,(t,k,r)] = enc^T[128k+p, 1024t+r]
  enc_first [128, 8*512]  fp16  block-0 first half, pre-tiled contiguous
  w_eT      [128, 4*8*128] fp16 w_eT[p, (j,k,oo)] = W_e[128j+oo, 128k+p]
  hb_in     [128, 4*4]    f32   hb_in[p,(j,b)] = (hidden @ W_h^T + bias)[b, 128j+p]
  v_pb      [128, 4]      f32   v_pb[p, j] = v[128j + p]
  v_pb16    [128, 4]      fp16  same, fp16 (final-block PE v-dot)
Output:
  probs     [4, 2048]     f32

The h-projection (hidden @ W_h^T + bias, 0.02% of total FLOPs) is
precomputed on host in exact f32 and shipped as a per-partition bias
table; everything else runs on device:
  E^T[o, r]  = sum_k W_e^T[k, o] encT[k, r]            (PE fp16, 8 k-chunks)
  et[o, r]   = tanh(E^T + hb[:, b])                    (ACT, per-partition bias)
  prod[o, r] = et * v[o]  summed over 4 o-chunks       (DVE mul/add tree, fp16)
  sc[r]      = ones^T @ prod                           (PE, K=128 -> [1, 512])
  expo       = exp(sc - 20), partial sums via accum_out (ACT, streaming softmax)
  probs[b,:] = expo / sum(expo)                        (DVE, per-batch finalize)

A run of warm-up matmuls on a memset tile covers the initial DMA window
so the PE HAM clock-gate is released (2.4 GHz) before the real stream
begins. DMA issue order is arranged so enc blocks are never queued
behind small transfers.
"""

import numpy as np

B = 32
S = 2048
DD = 512
DE2 = 1024
NCORES = 8
BL = B // NCORES  # 4 batches per core
R = BL * S  # 8192 rows per core
NK = DE2 // 128  # 8 k-chunks
NO = DD // 128  # 4 o-chunks
NB2 = R // 1024  # 8 DMA blocks of 1024 rows
EXP_OFF = -20.0  # softmax shift; scores observed in [-32, 27]
NWARM = 36

_CACHE = {}


def _build_bass():
    import concourse.bacc as bacc
    import concourse.mybir as mybir
    import concourse.tile as tile
    from concourse._compat import get_trn_type

    f32 = mybir.dt.float32
    f16 = mybir.dt.float16
    AF = mybir.ActivationFunctionType

    nc = bacc.Bacc(get_trn_type() or "TRN2", target_bir_lowering=False, debug=False)

    encB = nc.dram_tensor("encB", [128, NB2 * NK * 1024], f16, kind="ExternalInput")
    enc_first = nc.dram_tensor("enc_first", [128, NK * 512], f16, kind="ExternalInput")
    w_eT = nc.dram_tensor("w_eT", [128, NO * NK * 128], f16, kind="ExternalInput")
    hb_in = nc.dram_tensor("hb_in", [128, NO * BL], f32, kind="ExternalInput")
    v_pb = nc.dram_tensor("v_pb", [128, NO], f32, kind="ExternalInput")
    v_pb16 = nc.dram_tensor("v_pb16", [128, NO], f16, kind="ExternalInput")
    probs = nc.dram_tensor("probs", [BL, S], f32, kind="ExternalOutput")

    with tile.TileContext(nc) as tc:
        with (
            tc.tile_pool(name="const", bufs=1) as const,
            tc.tile_pool(name="encp", bufs=6) as encp,
            tc.tile_pool(name="etp", bufs=8) as etp,
            tc.tile_pool(name="prp", bufs=6) as prp,
            tc.tile_pool(name="pep", bufs=4, space="PSUM") as pep,
            tc.tile_pool(name="pmisc", bufs=2, space="PSUM") as pmisc,
            tc.tile_pool(name="pwu", bufs=1, space="PSUM") as pwu,
        ):
            # ---- PE warm-up: dummy matmuls while DMAs stream in ----
            warm_sb = const.tile([128, 128], f16, name="warm_sb")
            nc.any.memset(warm_sb[:], 0.0)
            wu_ps = pwu.tile([128, 128], f32, name="wu_ps", tag="wu")
            for i in range(NWARM):
                nc.tensor.matmul(
                    wu_ps[:], warm_sb[:], warm_sb[:], start=True, stop=True
                )

            # ---- critical-path DMAs, dual-issue: enc stream on Sync,
            # ---- weights/bias channel on Scalar (also HWDGE) ----
            encB_v = encB[:].rearrange("p (t k r) -> p t k r", t=NB2, k=NK)
            enc_first_v = enc_first[:].rearrange("p (k r) -> p k r", k=NK)
            b0h0 = const.tile([128, NK, 512], f16, name="b0h0")
            nc.sync.dma_start(b0h0[:, 0 : NK // 2], enc_first_v[:, 0 : NK // 2])
            nc.sync.dma_start(b0h0[:, NK // 2 :], enc_first_v[:, NK // 2 :])
            # we_sb[p, j, k, oo] = W_e[128j+oo, 128k+p]; per-j DMAs so the
            # first matmul group only waits on 256 KB of weights
            we_sb = const.tile([128, NO, NK, 128], f16, name="we_sb")
            we_v = w_eT[:].rearrange("p (j k oo) -> p j k oo", j=NO, k=NK)
            for j in range(NO):
                nc.scalar.dma_start(we_sb[:, j], we_v[:, j])
            hb_sb = const.tile([128, NO, BL], f32, name="hb_sb")
            nc.scalar.dma_start(hb_sb[:], hb_in[:].rearrange("p (j b) -> p j b", j=NO))
            v_sb = const.tile([128, NO], f32, name="v_sb")
            nc.scalar.dma_start(v_sb[:], v_pb[:])
            v16_sb = const.tile([128, NO], f16, name="v16_sb")
            nc.scalar.dma_start(v16_sb[:], v_pb16[:])
            b0h1 = const.tile([128, NK, 512], f16, name="b0h1")
            nc.sync.dma_start(b0h1[:], encB_v[:, 0, :, 512:1024])

            ones_v = const.tile([128, 1], f16, name="ones_v")
            nc.any.memset(ones_v[:], 1.0)
            expoff_sb = const.tile([1, 1], f32, name="expoff_sb")
            nc.any.memset(expoff_sb[:], EXP_OFF)
            expo_flat = const.tile([1, R], f32, name="expo_flat")
            sumparts = const.tile([1, 4 * BL], f32, name="sumparts")
            outp = const.tile([1, R], f32, name="outp")

            # ---- main loop: 8 DMA blocks x 2 halves of 512 rows ----
            for t2 in range(NB2):
                if t2 == 0:
                    halves = [b0h0, b0h1]
                else:
                    enc_t = encp.tile([128, NK, 1024], f16, name="enc_t", tag="enc")
                    nc.sync.dma_start(enc_t[:], encB_v[:, t2])
                    halves = [enc_t, enc_t]
                b = t2 // 2
                for h in range(2):
                    t_i = 2 * t2 + h  # 512-row block index, 4 per batch
                    last = t_i == 2 * NB2 - 1
                    src = halves[h]
                    lo = 0 if t2 == 0 else 512 * h
                    et_list = []
                    for j in range(NO):
                        pe = pep.tile([128, 512], f32, name="pe", tag="pe")
                        for k in range(NK):
                            nc.tensor.matmul(
                                pe[:],
                                we_sb[:, j, k, :],
                                src[:, k, lo : lo + 512],
                                start=(k == 0),
                                stop=(k == NK - 1),
                            )
                        et = etp.tile([128, 512], f16, name="et", tag="et")
                        nc.scalar.activation(
                            et[:], pe[:], AF.Tanh, bias=hb_sb[:, j, b : b + 1]
                        )
                        et_list.append(et)
                    sc = pmisc.tile([1, 512], f32, name="sc", tag="mi")
                    if last:
                        # final block: PE v-dot directly (shorter dep chain)
                        for j in range(NO):
                            nc.tensor.matmul(
                                sc[:],
                                v16_sb[:, j : j + 1],
                                et_list[j][:],
                                start=(j == 0),
                                stop=(j == NO - 1),
                            )
                    else:
                        # v-weighted sum over o: DVE tree + one K=128 ones-matmul
                        p01 = prp.tile([128, 512], f16, name="p01", tag="pr")
                        p23 = prp.tile([128, 512], f16, name="p23", tag="pr")
                        pa = prp.tile([128, 512], f16, name="pa", tag="pr")
                        nc.vector.tensor_scalar_mul(p01[:], et_list[0][:], v_sb[:, 0:1])
                        nc.vector.tensor_scalar_mul(pa[:], et_list[1][:], v_sb[:, 1:2])
                        nc.vector.tensor_add(p01[:], p01[:], pa[:])
                        nc.vector.tensor_scalar_mul(p23[:], et_list[2][:], v_sb[:, 2:3])
                        nc.vector.tensor_scalar_mul(pa[:], et_list[3][:], v_sb[:, 3:4])
                        nc.vector.tensor_add(p23[:], p23[:], pa[:])
                        nc.vector.tensor_add(p01[:], p01[:], p23[:])
                        nc.tensor.matmul(sc[:], ones_v[:], p01[:], start=True, stop=True)
                    # streaming softmax numerator + partial sum
                    nc.scalar.activation(
                        expo_flat[0:1, 512 * t_i : 512 * (t_i + 1)],
                        sc[:],
                        AF.Exp,
                        bias=expoff_sb[:],
                        accum_out=sumparts[0:1, t_i : t_i + 1],
                    )
                # ---- per-batch finalize once its 4 blocks are done ----
                if t2 % 2 == 1:
                    rsum = const.tile([1, 1], f32, name=f"rsum{b}", tag=f"rs{b}")
                    nc.vector.reduce_sum(
                        rsum[:],
                        sumparts[0:1, 4 * b : 4 * (b + 1)],
                        axis=mybir.AxisListType.X,
                    )
                    rec = const.tile([1, 1], f32, name=f"rec{b}", tag=f"rc{b}")
                    nc.vector.reciprocal(rec[:], rsum[:])
                    nc.vector.tensor_scalar_mul(
                        outp[0:1, S * b : S * (b + 1)],
                        expo_flat[0:1, S * b : S * (b + 1)],
                        rec[:],
                    )
                    nc.scalar.dma_start(
                        probs[b : b + 1, :], outp[0:1, S * b : S * (b + 1)]
                    )

    nc.compile()
    return nc


def _get_nc():
    if "nc" not in _CACHE:
        _CACHE["nc"] = _build_bass()
    return _CACHE["nc"]


def _tile_rows(mat_t, nchunk):
    # [nchunk*128, F] -> [128, nchunk*F] with out[p, c*F+f] = mat_t[128c+p, f]
    n, F = mat_t.shape
    assert n == nchunk * 128
    return np.ascontiguousarray(
        mat_t.reshape(nchunk, 128, F).transpose(1, 0, 2)
    ).reshape(128, nchunk * F)


def _make_in_maps(hidden, enc, W, b, v):
    W_h = W[:, :DD]
    W_e = W[:, DD:]
    # w_eT[p, j, k, oo] = W_e[128j+oo, 128k+p]
    w_eT = np.ascontiguousarray(
        W_e.reshape(NO, 128, NK, 128).transpose(3, 0, 2, 1)
    ).reshape(128, NO * NK * 128).astype(np.float16)
    v_pb = np.ascontiguousarray(v.reshape(NO, 128).T).astype(np.float32)
    v_pb16 = v_pb.astype(np.float16)
    enc16 = enc.astype(np.float16)  # [S, B, DE2]
    in_maps = []
    for c in range(NCORES):
        ec = enc16[:, BL * c : BL * (c + 1), :]  # [S, BL, DE2]
        encT = np.ascontiguousarray(ec.transpose(2, 1, 0)).reshape(DE2, R)
        # encB[p, t2, k, r] = encT[128k+p, 1024*t2 + r] (contiguous per block)
        encB = np.ascontiguousarray(
            encT.reshape(NK, 128, NB2, 1024).transpose(1, 2, 0, 3)
        ).reshape(128, NB2 * NK * 1024)
        enc_first = _tile_rows(np.ascontiguousarray(encT[:, :512]), NK)
        # exact f32 h-projection + bias, tiled per-partition: [128, (j, b)]
        h_proj = hidden[BL * c : BL * (c + 1), :] @ W_h.T + b  # [BL, DD]
        hb = _tile_rows(np.ascontiguousarray(h_proj.T), NO)  # [128, NO*BL]
        in_maps.append(
            {
                "encB": encB,
                "enc_first": enc_first,
                "w_eT": w_eT,
                "hb_in": np.ascontiguousarray(hb, dtype=np.float32),
                "v_pb": v_pb,
                "v_pb16": v_pb16,
            }
        )
    return in_maps


def kernel(hidden, encoder_outputs, W, b, v):
    """Full inputs in, full output out; 8-way batch-parallel inside."""
    from concourse.bass_utils import run_bass_kernel_spmd

    hidden = np.asarray(hidden, dtype=np.float32)
    enc = np.asarray(encoder_outputs, dtype=np.float32)
    W = np.asarray(W, dtype=np.float32)
    b = np.asarray(b, dtype=np.float32)
    v = np.asarray(v, dtype=np.float32)

    in_maps = _make_in_maps(hidden, enc, W, b, v)
    nc = _get_nc()
    res = run_bass_kernel_spmd(nc, in_maps, core_ids=list(range(NCORES)))
    out = np.concatenate([res.results[c]["probs"] for c in range(NCORES)], axis=0)
    return out.astype(np.float32)


# revision 37
# speedup vs baseline: 1.0242x; 1.0242x over previous
"""Bahdanau-attention scores kernel for Trainium2, 8-core data-parallel.

Computes softmax_s( v . tanh(W_h @ h[b] + W_e @ enc[s,b] + bias) ) for
B=32, S=2048, Dd=512, De2=1024, sharded 4 batches per NeuronCore.

Per-core device layout (host pre-shards / pre-tiles into per-partition
form so every DMA is 128 long contiguous runs; r = b_local*2048 + s):
  encB      [128, 8*8*1024] fp16 encB[p,(t,k,r)] = enc^T[128k+p, 1024t+r]
  enc_first [128, 8*512]  fp16  block-0 first half, pre-tiled contiguous
  w_eT      [128, 4*8*128] fp16 w_eT[p, (j,k,oo)] = W_e[128j+oo, 128k+p]
  hb_in     [128, 4*4]    f32   hb_in[p,(j,b)] = (hidden @ W_h^T + bias)[b, 128j+p]
  v_pb      [128, 4]      f32   v_pb[p, j] = v[128j + p]
  v_pb16    [128, 4]      fp16  same, fp16 (final-block PE v-dot)
Output:
  probs     [4, 2048]     f32

The h-projection (hidden @ W_h^T + bias, 0.02% of total FLOPs) is
precomputed on host in exact f32 and shipped as a per-partition bias
table; everything else runs on device:
  E^T[o, r]  = sum_k W_e^T[k, o] encT[k, r]            (PE fp16, 8 k-chunks)
  et[o, r]   = tanh(E^T + hb[:, b])                    (ACT, per-partition bias)
  prod[o, r] = et * v[o]  summed over 4 o-chunks       (DVE mul/add tree, fp16)
  sc[r]      = ones^T @ prod                           (PE, K=128 -> [1, 512])
  expo       = exp(sc - 20), partial sums via accum_out (ACT, streaming softmax)
  probs[b,:] = expo / sum(expo)                        (DVE, per-batch finalize)

A run of warm-up matmuls on a memset tile covers the initial DMA window
so the PE HAM clock-gate is released (2.4 GHz) before the real stream
begins. DMA issue order is arranged so enc blocks are never queued
behind small transfers.
"""

import numpy as np

B = 32
S = 2048
DD = 512
DE2 = 1024
NCORES = 8
BL = B // NCORES  # 4 batches per core
R = BL * S  # 8192 rows per core
NK = DE2 // 128  # 8 k-chunks
NO = DD // 128  # 4 o-chunks
NB2 = R // 1024  # 8 DMA blocks of 1024 rows
EXP_OFF = -20.0  # softmax shift; scores observed in [-32, 27]
NWARM = 88

_CACHE = {}


def _build_bass():
    import concourse.bacc as bacc
    import concourse.mybir as mybir
    import concourse.tile as tile
    from concourse._compat import get_trn_type

    f32 = mybir.dt.float32
    f16 = mybir.dt.float16
    AF = mybir.ActivationFunctionType

    nc = bacc.Bacc(get_trn_type() or "TRN2", target_bir_lowering=False, debug=False)

    encB = nc.dram_tensor("encB", [128, NB2 * NK * 1024], f16, kind="ExternalInput")
    enc_first = nc.dram_tensor("enc_first", [128, NK * 512], f16, kind="ExternalInput")
    w_eT = nc.dram_tensor("w_eT", [128, NO * NK * 128], f16, kind="ExternalInput")
    hb_in = nc.dram_tensor("hb_in", [128, NO * BL], f32, kind="ExternalInput")
    v_pb = nc.dram_tensor("v_pb", [128, NO], f32, kind="ExternalInput")
    v_pb16 = nc.dram_tensor("v_pb16", [128, NO], f16, kind="ExternalInput")
    probs = nc.dram_tensor("probs", [BL, S], f32, kind="ExternalOutput")

    with tile.TileContext(nc) as tc:
        with (
            tc.tile_pool(name="const", bufs=1) as const,
            tc.tile_pool(name="encp", bufs=6) as encp,
            tc.tile_pool(name="etp", bufs=8) as etp,
            tc.tile_pool(name="prp", bufs=6) as prp,
            tc.tile_pool(name="pep", bufs=4, space="PSUM") as pep,
            tc.tile_pool(name="pmisc", bufs=2, space="PSUM") as pmisc,
            tc.tile_pool(name="pwu", bufs=1, space="PSUM") as pwu,
        ):
            # ---- PE warm-up: dummy matmuls while DMAs stream in ----
            warm_sb = const.tile([128, 128], f16, name="warm_sb")
            nc.any.memset(warm_sb[:], 0.0)
            wu_ps = pwu.tile([128, 128], f32, name="wu_ps", tag="wu")
            for i in range(NWARM):
                nc.tensor.matmul(
                    wu_ps[:], warm_sb[:], warm_sb[:], start=True, stop=True
                )

            # ---- critical-path DMAs, dual-issue: enc stream on Sync,
            # ---- weights/bias channel on Scalar (also HWDGE) ----
            encB_v = encB[:].rearrange("p (t k r) -> p t k r", t=NB2, k=NK)
            enc_first_v = enc_first[:].rearrange("p (k r) -> p k r", k=NK)
            b0h0 = const.tile([128, NK, 512], f16, name="b0h0")
            nc.sync.dma_start(b0h0[:], enc_first_v[:])
            # we_sb[p, j, k, oo] = W_e[128j+oo, 128k+p]; per-j DMAs so the
            # first matmul group only waits on 256 KB of weights
            we_sb = const.tile([128, NO, NK, 128], f16, name="we_sb")
            we_v = w_eT[:].rearrange("p (j k oo) -> p j k oo", j=NO, k=NK)
            for j in range(NO):
                nc.scalar.dma_start(we_sb[:, j], we_v[:, j])
            hb_sb = const.tile([128, NO, BL], f32, name="hb_sb")
            nc.scalar.dma_start(hb_sb[:], hb_in[:].rearrange("p (j b) -> p j b", j=NO))
            v_sb = const.tile([128, NO], f32, name="v_sb")
            nc.scalar.dma_start(v_sb[:], v_pb[:])
            v16_sb = const.tile([128, NO], f16, name="v16_sb")
            nc.scalar.dma_start(v16_sb[:], v_pb16[:])
            b0h1 = const.tile([128, NK, 512], f16, name="b0h1")
            nc.sync.dma_start(b0h1[:], encB_v[:, 0, :, 512:1024])

            ones_v = const.tile([128, 1], f16, name="ones_v")
            nc.any.memset(ones_v[:], 1.0)
            expoff_sb = const.tile([1, 1], f32, name="expoff_sb")
            nc.any.memset(expoff_sb[:], EXP_OFF)
            expo_flat = const.tile([1, R], f32, name="expo_flat")
            sumparts = const.tile([1, 4 * BL], f32, name="sumparts")
            outp = const.tile([1, R], f32, name="outp")

            # ---- main loop: 8 DMA blocks x 2 halves of 512 rows ----
            for t2 in range(NB2):
                if t2 == 0:
                    halves = [b0h0, b0h1]
                else:
                    enc_t = encp.tile([128, NK, 1024], f16, name="enc_t", tag="enc")
                    nc.sync.dma_start(enc_t[:], encB_v[:, t2])
                    halves = [enc_t, enc_t]
                b = t2 // 2
                for h in range(2):
                    t_i = 2 * t2 + h  # 512-row block index, 4 per batch
                    last = t_i == 2 * NB2 - 1
                    src = halves[h]
                    lo = 0 if t2 == 0 else 512 * h
                    et_list = []
                    for j in range(NO):
                        pe = pep.tile([128, 512], f32, name="pe", tag="pe")
                        for k in range(NK):
                            nc.tensor.matmul(
                                pe[:],
                                we_sb[:, j, k, :],
                                src[:, k, lo : lo + 512],
                                start=(k == 0),
                                stop=(k == NK - 1),
                            )
                        et = etp.tile([128, 512], f16, name="et", tag="et")
                        nc.scalar.activation(
                            et[:], pe[:], AF.Tanh, bias=hb_sb[:, j, b : b + 1]
                        )
                        et_list.append(et)
                    sc = pmisc.tile([1, 512], f32, name="sc", tag="mi")
                    if last:
                        # final block: PE v-dot directly (shorter dep chain)
                        for j in range(NO):
                            nc.tensor.matmul(
                                sc[:],
                                v16_sb[:, j : j + 1],
                                et_list[j][:],
                                start=(j == 0),
                                stop=(j == NO - 1),
                            )
                    else:
                        # v-weighted sum over o: DVE tree + one K=128 ones-matmul
                        p01 = prp.tile([128, 512], f16, name="p01", tag="pr")
                        p23 = prp.tile([128, 512], f16, name="p23", tag="pr")
                        pa = prp.tile([128, 512], f16, name="pa", tag="pr")
                        nc.vector.tensor_scalar_mul(p01[:], et_list[0][:], v_sb[:, 0:1])
                        nc.vector.tensor_scalar_mul(pa[:], et_list[1][:], v_sb[:, 1:2])
                        nc.vector.tensor_add(p01[:], p01[:], pa[:])
                        nc.vector.tensor_scalar_mul(p23[:], et_list[2][:], v_sb[:, 2:3])
                        nc.vector.tensor_scalar_mul(pa[:], et_list[3][:], v_sb[:, 3:4])
                        nc.vector.tensor_add(p23[:], p23[:], pa[:])
                        nc.vector.tensor_add(p01[:], p01[:], p23[:])
                        nc.tensor.matmul(sc[:], ones_v[:], p01[:], start=True, stop=True)
                    # streaming softmax numerator + partial sum
                    nc.scalar.activation(
                        expo_flat[0:1, 512 * t_i : 512 * (t_i + 1)],
                        sc[:],
                        AF.Exp,
                        bias=expoff_sb[:],
                        accum_out=sumparts[0:1, t_i : t_i + 1],
                    )
                # ---- per-batch finalize once its 4 blocks are done ----
                if t2 % 2 == 1:
                    rsum = const.tile([1, 1], f32, name=f"rsum{b}", tag=f"rs{b}")
                    nc.vector.reduce_sum(
                        rsum[:],
                        sumparts[0:1, 4 * b : 4 * (b + 1)],
                        axis=mybir.AxisListType.X,
                    )
                    rec = const.tile([1, 1], f32, name=f"rec{b}", tag=f"rc{b}")
                    nc.vector.reciprocal(rec[:], rsum[:])
                    nc.vector.tensor_scalar_mul(
                        outp[0:1, S * b : S * (b + 1)],
                        expo_flat[0:1, S * b : S * (b + 1)],
                        rec[:],
                    )
                    nc.scalar.dma_start(
                        probs[b : b + 1, :], outp[0:1, S * b : S * (b + 1)]
                    )

    nc.compile()
    return nc


def _get_nc():
    if "nc" not in _CACHE:
        _CACHE["nc"] = _build_bass()
    return _CACHE["nc"]


def _tile_rows(mat_t, nchunk):
    # [nchunk*128, F] -> [128, nchunk*F] with out[p, c*F+f] = mat_t[128c+p, f]
    n, F = mat_t.shape
    assert n == nchunk * 128
    return np.ascontiguousarray(
        mat_t.reshape(nchunk, 128, F).transpose(1, 0, 2)
    ).reshape(128, nchunk * F)


def _make_in_maps(hidden, enc, W, b, v):
    W_h = W[:, :DD]
    W_e = W[:, DD:]
    # w_eT[p, j, k, oo] = W_e[128j+oo, 128k+p]
    w_eT = np.ascontiguousarray(
        W_e.reshape(NO, 128, NK, 128).transpose(3, 0, 2, 1)
    ).reshape(128, NO * NK * 128).astype(np.float16)
    v_pb = np.ascontiguousarray(v.reshape(NO, 128).T).astype(np.float32)
    v_pb16 = v_pb.astype(np.float16)
    enc16 = enc.astype(np.float16)  # [S, B, DE2]
    in_maps = []
    for c in range(NCORES):
        ec = enc16[:, BL * c : BL * (c + 1), :]  # [S, BL, DE2]
        encT = np.ascontiguousarray(ec.transpose(2, 1, 0)).reshape(DE2, R)
        # encB[p, t2, k, r] = encT[128k+p, 1024*t2 + r] (contiguous per block)
        encB = np.ascontiguousarray(
            encT.reshape(NK, 128, NB2, 1024).transpose(1, 2, 0, 3)
        ).reshape(128, NB2 * NK * 1024)
        enc_first = _tile_rows(np.ascontiguousarray(encT[:, :512]), NK)
        # exact f32 h-projection + bias, tiled per-partition: [128, (j, b)]
        h_proj = hidden[BL * c : BL * (c + 1), :] @ W_h.T + b  # [BL, DD]
        hb = _tile_rows(np.ascontiguousarray(h_proj.T), NO)  # [128, NO*BL]
        in_maps.append(
            {
                "encB": encB,
                "enc_first": enc_first,
                "w_eT": w_eT,
                "hb_in": np.ascontiguousarray(hb, dtype=np.float32),
                "v_pb": v_pb,
                "v_pb16": v_pb16,
            }
        )
    return in_maps


def kernel(hidden, encoder_outputs, W, b, v):
    """Full inputs in, full output out; 8-way batch-parallel inside."""
    from concourse.bass_utils import run_bass_kernel_spmd

    hidden = np.asarray(hidden, dtype=np.float32)
    enc = np.asarray(encoder_outputs, dtype=np.float32)
    W = np.asarray(W, dtype=np.float32)
    b = np.asarray(b, dtype=np.float32)
    v = np.asarray(v, dtype=np.float32)

    in_maps = _make_in_maps(hidden, enc, W, b, v)
    nc = _get_nc()
    res = run_bass_kernel_spmd(nc, in_maps, core_ids=list(range(NCORES)))
    out = np.concatenate([res.results[c]["probs"] for c in range(NCORES)], axis=0)
    return out.astype(np.float32)


# revision 38
# speedup vs baseline: 1.0295x; 1.0052x over previous
"""Bahdanau-attention scores kernel for Trainium2, 8-core data-parallel.

Computes softmax_s( v . tanh(W_h @ h[b] + W_e @ enc[s,b] + bias) ) for
B=32, S=2048, Dd=512, De2=1024, sharded 4 batches per NeuronCore.

Per-core device layout (host pre-shards / pre-tiles into per-partition
form so every DMA is 128 long contiguous runs; r = b_local*2048 + s):
  encB      [128, 8*8*1024] fp16 encB[p,(t,k,r)] = enc^T[128k+p, 1024t+r]
  enc_first [128, 8*512]  fp16  block-0 first half, pre-tiled contiguous
  w_eT      [128, 4*8*128] fp16 w_eT[p, (j,k,oo)] = W_e[128j+oo, 128k+p]
  hb_in     [128, 4*4]    f32   hb_in[p,(j,b)] = (hidden @ W_h^T + bias)[b, 128j+p]
  v_pb      [128, 4]      f32   v_pb[p, j] = v[128j + p]
  v_pb16    [128, 4]      fp16  same, fp16 (final-block PE v-dot)
Output:
  probs     [4, 2048]     f32

The h-projection (hidden @ W_h^T + bias, 0.02% of total FLOPs) is
precomputed on host in exact f32 and shipped as a per-partition bias
table; everything else runs on device:
  E^T[o, r]  = sum_k W_e^T[k, o] encT[k, r]            (PE fp16, 8 k-chunks)
  et[o, r]   = tanh(E^T + hb[:, b])                    (ACT, per-partition bias)
  prod[o, r] = et * v[o]  summed over 4 o-chunks       (DVE mul/add tree, fp16)
  sc[r]      = ones^T @ prod                           (PE, K=128 -> [1, 512])
  expo       = exp(sc - 20), partial sums via accum_out (ACT, streaming softmax)
  probs[b,:] = expo / sum(expo)                        (DVE, per-batch finalize)

A run of warm-up matmuls on a memset tile covers the initial DMA window
so the PE HAM clock-gate is released (2.4 GHz) before the real stream
begins. DMA issue order is arranged so enc blocks are never queued
behind small transfers.
"""

import numpy as np

B = 32
S = 2048
DD = 512
DE2 = 1024
NCORES = 8
BL = B // NCORES  # 4 batches per core
R = BL * S  # 8192 rows per core
NK = DE2 // 128  # 8 k-chunks
NO = DD // 128  # 4 o-chunks
NB2 = R // 1024  # 8 DMA blocks of 1024 rows
EXP_OFF = -20.0  # softmax shift; scores observed in [-32, 27]
NWARM = 88

_CACHE = {}


def _build_bass():
    import concourse.bacc as bacc
    import concourse.mybir as mybir
    import concourse.tile as tile
    from concourse._compat import get_trn_type

    f32 = mybir.dt.float32
    f16 = mybir.dt.float16
    AF = mybir.ActivationFunctionType

    nc = bacc.Bacc(get_trn_type() or "TRN2", target_bir_lowering=False, debug=False)

    encB = nc.dram_tensor("encB", [128, NB2 * NK * 1024], f16, kind="ExternalInput")
    enc_first = nc.dram_tensor("enc_first", [128, NK * 512], f16, kind="ExternalInput")
    w_eT = nc.dram_tensor("w_eT", [128, NO * NK * 128], f16, kind="ExternalInput")
    hb_in = nc.dram_tensor("hb_in", [128, NO * BL], f32, kind="ExternalInput")
    v_pb = nc.dram_tensor("v_pb", [128, NO], f32, kind="ExternalInput")
    v_pb16 = nc.dram_tensor("v_pb16", [128, NO], f16, kind="ExternalInput")
    probs = nc.dram_tensor("probs", [BL, S], f32, kind="ExternalOutput")

    with tile.TileContext(nc) as tc:
        with (
            tc.tile_pool(name="const", bufs=1) as const,
            tc.tile_pool(name="encp", bufs=6) as encp,
            tc.tile_pool(name="etp", bufs=8) as etp,
            tc.tile_pool(name="prp", bufs=6) as prp,
            tc.tile_pool(name="pep", bufs=4, space="PSUM") as pep,
            tc.tile_pool(name="pmisc", bufs=2, space="PSUM") as pmisc,
            tc.tile_pool(name="pwu", bufs=1, space="PSUM") as pwu,
        ):
            # ---- PE warm-up: dummy matmuls while DMAs stream in ----
            warm_sb = const.tile([128, 128], f16, name="warm_sb")
            nc.any.memset(warm_sb[:], 0.0)
            wu_ps = pwu.tile([128, 128], f32, name="wu_ps", tag="wu")
            for i in range(NWARM):
                nc.tensor.matmul(
                    wu_ps[:], warm_sb[:], warm_sb[:], start=True, stop=True
                )

            # ---- critical-path DMAs, dual-issue: enc stream on Sync,
            # ---- weights/bias channel on Scalar (also HWDGE) ----
            encB_v = encB[:].rearrange("p (t k r) -> p t k r", t=NB2, k=NK)
            enc_first_v = enc_first[:].rearrange("p (k r) -> p k r", k=NK)
            b0h0 = const.tile([128, NK, 512], f16, name="b0h0")
            nc.sync.dma_start(b0h0[:], enc_first_v[:])
            # we_sb[p, j, k, oo] = W_e[128j+oo, 128k+p]; per-j DMAs so the
            # first matmul group only waits on 256 KB of weights
            we_sb = const.tile([128, NO, NK, 128], f16, name="we_sb")
            we_v = w_eT[:].rearrange("p (j k oo) -> p j k oo", j=NO, k=NK)
            for j in range(NO):
                nc.scalar.dma_start(we_sb[:, j], we_v[:, j])
            hb_sb = const.tile([128, NO, BL], f32, name="hb_sb")
            nc.scalar.dma_start(hb_sb[:], hb_in[:].rearrange("p (j b) -> p j b", j=NO))
            v_sb = const.tile([128, NO], f32, name="v_sb")
            nc.scalar.dma_start(v_sb[:], v_pb[:])
            v16_sb = const.tile([128, NO], f16, name="v16_sb")
            nc.scalar.dma_start(v16_sb[:], v_pb16[:])
            b0h1 = const.tile([128, NK, 512], f16, name="b0h1")
            nc.sync.dma_start(b0h1[:], encB_v[:, 0, :, 512:1024])

            ones_v = const.tile([128, 1], f16, name="ones_v")
            nc.any.memset(ones_v[:], 1.0)
            expoff_sb = const.tile([1, 1], f32, name="expoff_sb")
            nc.any.memset(expoff_sb[:], EXP_OFF)
            expo_flat = const.tile([1, R], f32, name="expo_flat")
            sumparts = const.tile([1, 4 * BL], f32, name="sumparts")
            outp = const.tile([1, R], f32, name="outp")

            # ---- main loop: 8 DMA blocks x 2 halves of 512 rows ----
            for t2 in range(NB2):
                if t2 == 0:
                    halves = [b0h0, b0h1]
                else:
                    enc_t = encp.tile([128, NK, 1024], f16, name="enc_t", tag="enc")
                    nc.sync.dma_start(enc_t[:], encB_v[:, t2])
                    halves = [enc_t, enc_t]
                b = t2 // 2
                for h in range(2):
                    t_i = 2 * t2 + h  # 512-row block index, 4 per batch
                    last = t_i == 2 * NB2 - 1
                    src = halves[h]
                    lo = 0 if t2 == 0 else 512 * h
                    et_list = []
                    for j in range(NO):
                        pe = pep.tile([128, 512], f32, name="pe", tag="pe")
                        for k in range(NK):
                            nc.tensor.matmul(
                                pe[:],
                                we_sb[:, j, k, :],
                                src[:, k, lo : lo + 512],
                                start=(k == 0),
                                stop=(k == NK - 1),
                            )
                        et = etp.tile([128, 512], f16, name="et", tag="et")
                        nc.scalar.activation(
                            et[:], pe[:], AF.Tanh, bias=hb_sb[:, j, b : b + 1]
                        )
                        et_list.append(et)
                    sc = pmisc.tile([1, 512], f32, name="sc", tag="mi")
                    if last:
                        # final block: PE v-dot directly (shorter dep chain)
                        for j in range(NO):
                            nc.tensor.matmul(
                                sc[:],
                                v16_sb[:, j : j + 1],
                                et_list[j][:],
                                start=(j == 0),
                                stop=(j == NO - 1),
                            )
                    else:
                        # v-weighted sum over o: DVE tree + one K=128 ones-matmul
                        p01 = prp.tile([128, 512], f16, name="p01", tag="pr")
                        p23 = prp.tile([128, 512], f16, name="p23", tag="pr")
                        pa = prp.tile([128, 512], f16, name="pa", tag="pr")
                        nc.vector.tensor_scalar_mul(p01[:], et_list[0][:], v_sb[:, 0:1])
                        nc.vector.tensor_scalar_mul(pa[:], et_list[1][:], v_sb[:, 1:2])
                        nc.vector.tensor_add(p01[:], p01[:], pa[:])
                        nc.vector.tensor_scalar_mul(p23[:], et_list[2][:], v_sb[:, 2:3])
                        nc.vector.tensor_scalar_mul(pa[:], et_list[3][:], v_sb[:, 3:4])
                        nc.vector.tensor_add(p23[:], p23[:], pa[:])
                        nc.vector.tensor_add(p01[:], p01[:], p23[:])
                        nc.tensor.matmul(sc[:], ones_v[:], p01[:], start=True, stop=True)
                    # streaming softmax numerator + partial sum
                    nc.scalar.activation(
                        expo_flat[0:1, 512 * t_i : 512 * (t_i + 1)],
                        sc[:],
                        AF.Exp,
                        bias=expoff_sb[:],
                        accum_out=sumparts[0:1, t_i : t_i + 1],
                    )
                # ---- per-batch finalize once its 4 blocks are done ----
                if t2 % 2 == 1:
                    rsum = const.tile([1, 1], f32, name=f"rsum{b}", tag=f"rs{b}")
                    nc.vector.reduce_sum(
                        rsum[:],
                        sumparts[0:1, 4 * b : 4 * (b + 1)],
                        axis=mybir.AxisListType.X,
                    )
                    rec = const.tile([1, 1], f32, name=f"rec{b}", tag=f"rc{b}")
                    nc.vector.reciprocal(rec[:], rsum[:])
                    # halves: first store overlaps the second half's multiply
                    for u in range(2):
                        lo2 = S * b + (S // 2) * u
                        hi2 = lo2 + S // 2
                        nc.vector.tensor_scalar_mul(
                            outp[0:1, lo2:hi2], expo_flat[0:1, lo2:hi2], rec[:]
                        )
                        nc.scalar.dma_start(
                            probs[b : b + 1, (S // 2) * u : (S // 2) * (u + 1)],
                            outp[0:1, lo2:hi2],
                        )

    nc.compile()
    return nc


def _get_nc():
    if "nc" not in _CACHE:
        _CACHE["nc"] = _build_bass()
    return _CACHE["nc"]


def _tile_rows(mat_t, nchunk):
    # [nchunk*128, F] -> [128, nchunk*F] with out[p, c*F+f] = mat_t[128c+p, f]
    n, F = mat_t.shape
    assert n == nchunk * 128
    return np.ascontiguousarray(
        mat_t.reshape(nchunk, 128, F).transpose(1, 0, 2)
    ).reshape(128, nchunk * F)


def _make_in_maps(hidden, enc, W, b, v):
    W_h = W[:, :DD]
    W_e = W[:, DD:]
    # w_eT[p, j, k, oo] = W_e[128j+oo, 128k+p]
    w_eT = np.ascontiguousarray(
        W_e.reshape(NO, 128, NK, 128).transpose(3, 0, 2, 1)
    ).reshape(128, NO * NK * 128).astype(np.float16)
    v_pb = np.ascontiguousarray(v.reshape(NO, 128).T).astype(np.float32)
    v_pb16 = v_pb.astype(np.float16)
    enc16 = enc.astype(np.float16)  # [S, B, DE2]
    in_maps = []
    for c in range(NCORES):
        ec = enc16[:, BL * c : BL * (c + 1), :]  # [S, BL, DE2]
        encT = np.ascontiguousarray(ec.transpose(2, 1, 0)).reshape(DE2, R)
        # encB[p, t2, k, r] = encT[128k+p, 1024*t2 + r] (contiguous per block)
        encB = np.ascontiguousarray(
            encT.reshape(NK, 128, NB2, 1024).transpose(1, 2, 0, 3)
        ).reshape(128, NB2 * NK * 1024)
        enc_first = _tile_rows(np.ascontiguousarray(encT[:, :512]), NK)
        # exact f32 h-projection + bias, tiled per-partition: [128, (j, b)]
        h_proj = hidden[BL * c : BL * (c + 1), :] @ W_h.T + b  # [BL, DD]
        hb = _tile_rows(np.ascontiguousarray(h_proj.T), NO)  # [128, NO*BL]
        in_maps.append(
            {
                "encB": encB,
                "enc_first": enc_first,
                "w_eT": w_eT,
                "hb_in": np.ascontiguousarray(hb, dtype=np.float32),
                "v_pb": v_pb,
                "v_pb16": v_pb16,
            }
        )
    return in_maps


def kernel(hidden, encoder_outputs, W, b, v):
    """Full inputs in, full output out; 8-way batch-parallel inside."""
    from concourse.bass_utils import run_bass_kernel_spmd

    hidden = np.asarray(hidden, dtype=np.float32)
    enc = np.asarray(encoder_outputs, dtype=np.float32)
    W = np.asarray(W, dtype=np.float32)
    b = np.asarray(b, dtype=np.float32)
    v = np.asarray(v, dtype=np.float32)

    in_maps = _make_in_maps(hidden, enc, W, b, v)
    nc = _get_nc()
    res = run_bass_kernel_spmd(nc, in_maps, core_ids=list(range(NCORES)))
    out = np.concatenate([res.results[c]["probs"] for c in range(NCORES)], axis=0)
    return out.astype(np.float32)


# revision 40
# speedup vs baseline: 1.0330x; 1.0034x over previous
"""Bahdanau-attention scores kernel for Trainium2, 8-core data-parallel.

Computes softmax_s( v . tanh(W_h @ h[b] + W_e @ enc[s,b] + bias) ) for
B=32, S=2048, Dd=512, De2=1024, sharded 4 batches per NeuronCore.

Per-core device layout (host pre-shards / pre-tiles into per-partition
form so every DMA is 128 long contiguous runs; r = b_local*2048 + s):
  encB      [128, 8*8*1024] fp16 encB[p,(t,k,r)] = enc^T[128k+p, 1024t+r]
  enc_first [128, 8*512]  fp16  block-0 first half, pre-tiled contiguous
  w_eT      [128, 4*8*128] fp16 w_eT[p, (j,k,oo)] = W_e[128j+oo, 128k+p]
  hb_in     [128, 4*4]    f32   hb_in[p,(j,b)] = (hidden @ W_h^T + bias)[b, 128j+p]
  v_pb      [128, 4]      f32   v_pb[p, j] = v[128j + p]
  v_pb16    [128, 4]      fp16  same, fp16 (final-block PE v-dot)
Output:
  probs     [4, 2048]     f32

The h-projection (hidden @ W_h^T + bias, 0.02% of total FLOPs) is
precomputed on host in exact f32 and shipped as a per-partition bias
table; everything else runs on device:
  E^T[o, r]  = sum_k W_e^T[k, o] encT[k, r]            (PE fp16, 8 k-chunks)
  et[o, r]   = tanh(E^T + hb[:, b])                    (ACT, per-partition bias)
  prod[o, r] = et * v[o]  summed over 4 o-chunks       (DVE mul/add tree, fp16)
  sc[r]      = ones^T @ prod                           (PE, K=128 -> [1, 512])
  expo       = exp(sc - 20), partial sums via accum_out (ACT, streaming softmax)
  probs[b,:] = expo / sum(expo)                        (DVE, per-batch finalize)

A run of warm-up matmuls on a memset tile covers the initial DMA window
so the PE HAM clock-gate is released (2.4 GHz) before the real stream
begins. DMA issue order is arranged so enc blocks are never queued
behind small transfers.
"""

import numpy as np

B = 32
S = 2048
DD = 512
DE2 = 1024
NCORES = 8
BL = B // NCORES  # 4 batches per core
R = BL * S  # 8192 rows per core
NK = DE2 // 128  # 8 k-chunks
NO = DD // 128  # 4 o-chunks
NB2 = R // 1024  # 8 DMA blocks of 1024 rows
EXP_OFF = -20.0  # softmax shift; scores observed in [-32, 27]
NWARM = 88

_CACHE = {}


def _build_bass():
    import concourse.bacc as bacc
    import concourse.mybir as mybir
    import concourse.tile as tile
    from concourse._compat import get_trn_type

    f32 = mybir.dt.float32
    f16 = mybir.dt.float16
    AF = mybir.ActivationFunctionType

    nc = bacc.Bacc(get_trn_type() or "TRN2", target_bir_lowering=False, debug=False)

    encB = nc.dram_tensor("encB", [128, NB2 * NK * 1024], f16, kind="ExternalInput")
    enc_first = nc.dram_tensor("enc_first", [128, NK * 512], f16, kind="ExternalInput")
    w_eT = nc.dram_tensor("w_eT", [128, NO * NK * 128], f16, kind="ExternalInput")
    hb_in = nc.dram_tensor("hb_in", [128, NO * BL], f32, kind="ExternalInput")
    v_pb = nc.dram_tensor("v_pb", [128, NO], f32, kind="ExternalInput")
    v_pb16 = nc.dram_tensor("v_pb16", [128, NO], f16, kind="ExternalInput")
    probs = nc.dram_tensor("probs", [BL, S], f32, kind="ExternalOutput")

    with tile.TileContext(nc) as tc:
        with (
            tc.tile_pool(name="const", bufs=1) as const,
            tc.tile_pool(name="encp", bufs=6) as encp,
            tc.tile_pool(name="etp", bufs=8) as etp,
            tc.tile_pool(name="prp", bufs=10) as prp,
            tc.tile_pool(name="pep", bufs=4, space="PSUM") as pep,
            tc.tile_pool(name="pmisc", bufs=2, space="PSUM") as pmisc,
            tc.tile_pool(name="pwu", bufs=1, space="PSUM") as pwu,
        ):
            # ---- PE warm-up: dummy matmuls while DMAs stream in ----
            warm_sb = const.tile([128, 128], f16, name="warm_sb")
            nc.any.memset(warm_sb[:], 0.0)
            wu_ps = pwu.tile([128, 128], f32, name="wu_ps", tag="wu")
            for i in range(NWARM):
                nc.tensor.matmul(
                    wu_ps[:], warm_sb[:], warm_sb[:], start=True, stop=True
                )

            # ---- critical-path DMAs, dual-issue: enc stream on Sync,
            # ---- weights/bias channel on Scalar (also HWDGE) ----
            encB_v = encB[:].rearrange("p (t k r) -> p t k r", t=NB2, k=NK)
            enc_first_v = enc_first[:].rearrange("p (k r) -> p k r", k=NK)
            b0h0 = const.tile([128, NK, 512], f16, name="b0h0")
            nc.sync.dma_start(b0h0[:], enc_first_v[:])
            # we_sb[p, j, k, oo] = W_e[128j+oo, 128k+p]; per-j DMAs so the
            # first matmul group only waits on 256 KB of weights
            we_sb = const.tile([128, NO, NK, 128], f16, name="we_sb")
            we_v = w_eT[:].rearrange("p (j k oo) -> p j k oo", j=NO, k=NK)
            for j in range(NO):
                nc.scalar.dma_start(we_sb[:, j], we_v[:, j])
            hb_sb = const.tile([128, NO, BL], f32, name="hb_sb")
            nc.scalar.dma_start(hb_sb[:], hb_in[:].rearrange("p (j b) -> p j b", j=NO))
            v_sb = const.tile([128, NO], f32, name="v_sb")
            nc.scalar.dma_start(v_sb[:], v_pb[:])
            v16_sb = const.tile([128, NO], f16, name="v16_sb")
            nc.scalar.dma_start(v16_sb[:], v_pb16[:])
            b0h1 = const.tile([128, NK, 512], f16, name="b0h1")
            nc.sync.dma_start(b0h1[:], encB_v[:, 0, :, 512:1024])

            ones_v = const.tile([128, 1], f16, name="ones_v")
            nc.any.memset(ones_v[:], 1.0)
            expoff_sb = const.tile([1, 1], f32, name="expoff_sb")
            nc.any.memset(expoff_sb[:], EXP_OFF)
            expo_flat = const.tile([1, R], f32, name="expo_flat")
            sumparts = const.tile([1, 4 * BL], f32, name="sumparts")
            outp = const.tile([1, R], f32, name="outp")

            def emit_exp(sc, t_i):
                # streaming softmax numerator + partial sum
                nc.scalar.activation(
                    expo_flat[0:1, 512 * t_i : 512 * (t_i + 1)],
                    sc[:],
                    AF.Exp,
                    bias=expoff_sb[:],
                    accum_out=sumparts[0:1, t_i : t_i + 1],
                )

            def emit_finalize(b):
                rsum = const.tile([1, 1], f32, name=f"rsum{b}", tag=f"rs{b}")
                nc.vector.reduce_sum(
                    rsum[:],
                    sumparts[0:1, 4 * b : 4 * (b + 1)],
                    axis=mybir.AxisListType.X,
                )
                rec = const.tile([1, 1], f32, name=f"rec{b}", tag=f"rc{b}")
                nc.vector.reciprocal(rec[:], rsum[:])
                # halves: first store overlaps the second half's multiply
                for u in range(2):
                    lo2 = S * b + (S // 2) * u
                    hi2 = lo2 + S // 2
                    nc.vector.tensor_scalar_mul(
                        outp[0:1, lo2:hi2], expo_flat[0:1, lo2:hi2], rec[:]
                    )
                    nc.scalar.dma_start(
                        probs[b : b + 1, (S // 2) * u : (S // 2) * (u + 1)],
                        outp[0:1, lo2:hi2],
                    )

            def emit_score(st):
                # ones-matmul deferred one half-block: its DVE-tree input is
                # long finished, so the PE never stalls on sem 157
                p0, b0_, t0_ = st
                sc = pmisc.tile([1, 512], f32, name="sc", tag="mi")
                nc.tensor.matmul(sc[:], ones_v[:], p0[:], start=True, stop=True)
                emit_exp(sc, t0_)
                if t0_ % 4 == 3:
                    emit_finalize(b0_)

            # ---- main loop: 8 DMA blocks x 2 halves of 512 rows ----
            pending = None
            for t2 in range(NB2):
                if t2 == 0:
                    halves = [b0h0, b0h1]
                else:
                    enc_t = encp.tile([128, NK, 1024], f16, name="enc_t", tag="enc")
                    nc.sync.dma_start(enc_t[:], encB_v[:, t2])
                    halves = [enc_t, enc_t]
                b = t2 // 2
                for h in range(2):
                    t_i = 2 * t2 + h  # 512-row block index, 4 per batch
                    last = t_i == 2 * NB2 - 1
                    src = halves[h]
                    lo = 0 if t2 == 0 else 512 * h
                    et_list = []
                    prods = []
                    for j in range(NO):
                        pe = pep.tile([128, 512], f32, name="pe", tag="pe")
                        for k in range(NK):
                            nc.tensor.matmul(
                                pe[:],
                                we_sb[:, j, k, :],
                                src[:, k, lo : lo + 512],
                                start=(k == 0),
                                stop=(k == NK - 1),
                            )
                        et = etp.tile([128, 512], f16, name="et", tag="et")
                        nc.scalar.activation(
                            et[:], pe[:], AF.Tanh, bias=hb_sb[:, j, b : b + 1]
                        )
                        et_list.append(et)
                        if not last:
                            # v-scale immediately so the DVE tree tracks the tanhs
                            pj = prp.tile([128, 512], f16, name=f"pj{j}", tag="pr")
                            nc.vector.tensor_scalar_mul(
                                pj[:], et[:], v_sb[:, j : j + 1]
                            )
                            prods.append(pj)
                    if last:
                        # final block: PE v-dot directly (shortest dep chain),
                        # after flushing the previous block's deferred score
                        if pending is not None:
                            emit_score(pending)
                            pending = None
                        sc = pmisc.tile([1, 512], f32, name="sc", tag="mi")
                        for j in range(NO):
                            nc.tensor.matmul(
                                sc[:],
                                v16_sb[:, j : j + 1],
                                et_list[j][:],
                                start=(j == 0),
                                stop=(j == NO - 1),
                            )
                        emit_exp(sc, t_i)
                        emit_finalize(b)
                    else:
                        nc.vector.tensor_add(prods[0][:], prods[0][:], prods[1][:])
                        nc.vector.tensor_add(prods[2][:], prods[2][:], prods[3][:])
                        nc.vector.tensor_add(prods[0][:], prods[0][:], prods[2][:])
                        if pending is not None:
                            emit_score(pending)
                        pending = (prods[0], b, t_i)

    nc.compile()
    return nc


def _get_nc():
    if "nc" not in _CACHE:
        _CACHE["nc"] = _build_bass()
    return _CACHE["nc"]


def _tile_rows(mat_t, nchunk):
    # [nchunk*128, F] -> [128, nchunk*F] with out[p, c*F+f] = mat_t[128c+p, f]
    n, F = mat_t.shape
    assert n == nchunk * 128
    return np.ascontiguousarray(
        mat_t.reshape(nchunk, 128, F).transpose(1, 0, 2)
    ).reshape(128, nchunk * F)


def _make_in_maps(hidden, enc, W, b, v):
    W_h = W[:, :DD]
    W_e = W[:, DD:]
    # w_eT[p, j, k, oo] = W_e[128j+oo, 128k+p]
    w_eT = np.ascontiguousarray(
        W_e.reshape(NO, 128, NK, 128).transpose(3, 0, 2, 1)
    ).reshape(128, NO * NK * 128).astype(np.float16)
    v_pb = np.ascontiguousarray(v.reshape(NO, 128).T).astype(np.float32)
    v_pb16 = v_pb.astype(np.float16)
    enc16 = enc.astype(np.float16)  # [S, B, DE2]
    in_maps = []
    for c in range(NCORES):
        ec = enc16[:, BL * c : BL * (c + 1), :]  # [S, BL, DE2]
        encT = np.ascontiguousarray(ec.transpose(2, 1, 0)).reshape(DE2, R)
        # encB[p, t2, k, r] = encT[128k+p, 1024*t2 + r] (contiguous per block)
        encB = np.ascontiguousarray(
            encT.reshape(NK, 128, NB2, 1024).transpose(1, 2, 0, 3)
        ).reshape(128, NB2 * NK * 1024)
        enc_first = _tile_rows(np.ascontiguousarray(encT[:, :512]), NK)
        # exact f32 h-projection + bias, tiled per-partition: [128, (j, b)]
        h_proj = hidden[BL * c : BL * (c + 1), :] @ W_h.T + b  # [BL, DD]
        hb = _tile_rows(np.ascontiguousarray(h_proj.T), NO)  # [128, NO*BL]
        in_maps.append(
            {
                "encB": encB,
                "enc_first": enc_first,
                "w_eT": w_eT,
                "hb_in": np.ascontiguousarray(hb, dtype=np.float32),
                "v_pb": v_pb,
                "v_pb16": v_pb16,
            }
        )
    return in_maps


def kernel(hidden, encoder_outputs, W, b, v):
    """Full inputs in, full output out; 8-way batch-parallel inside."""
    from concourse.bass_utils import run_bass_kernel_spmd

    hidden = np.asarray(hidden, dtype=np.float32)
    enc = np.asarray(encoder_outputs, dtype=np.float32)
    W = np.asarray(W, dtype=np.float32)
    b = np.asarray(b, dtype=np.float32)
    v = np.asarray(v, dtype=np.float32)

    in_maps = _make_in_maps(hidden, enc, W, b, v)
    nc = _get_nc()
    res = run_bass_kernel_spmd(nc, in_maps, core_ids=list(range(NCORES)))
    out = np.concatenate([res.results[c]["probs"] for c in range(NCORES)], axis=0)
    return out.astype(np.float32)


# revision 43
# speedup vs baseline: 1.0332x; 1.0001x over previous
"""Bahdanau-attention scores kernel for Trainium2, 8-core data-parallel.

Computes softmax_s( v . tanh(W_h @ h[b] + W_e @ enc[s,b] + bias) ) for
B=32, S=2048, Dd=512, De2=1024, sharded 4 batches per NeuronCore.

Per-core device layout (host pre-shards / pre-tiles into per-partition
form so every DMA is 128 long contiguous runs; r = b_local*2048 + s):
  encB      [128, 8*8*1024] fp16 encB[p,(t,k,r)] = enc^T[128k+p, 1024t+r]
  enc_first [128, 8*512]  fp16  block-0 first half, pre-tiled contiguous
  w_eT      [128, 4*8*128] fp16 w_eT[p, (j,k,oo)] = W_e[128j+oo, 128k+p]
  hb_in     [128, 4*4]    f32   hb_in[p,(j,b)] = (hidden @ W_h^T + bias)[b, 128j+p]
  v_pb      [128, 4]      f32   v_pb[p, j] = v[128j + p]
  v_pb16    [128, 4]      fp16  same, fp16 (final-block PE v-dot)
Output:
  probs     [4, 2048]     f32

The h-projection (hidden @ W_h^T + bias, 0.02% of total FLOPs) is
precomputed on host in exact f32 and shipped as a per-partition bias
table; everything else runs on device:
  E^T[o, r]  = sum_k W_e^T[k, o] encT[k, r]            (PE fp16, 8 k-chunks)
  et[o, r]   = tanh(E^T + hb[:, b])                    (ACT, per-partition bias)
  prod[o, r] = et * v[o]  summed over 4 o-chunks       (DVE mul/add tree, fp16)
  sc[r]      = ones^T @ prod                           (PE, K=128 -> [1, 512])
  expo       = exp(sc - 20), partial sums via accum_out (ACT, streaming softmax)
  probs[b,:] = expo / sum(expo)                        (DVE, per-batch finalize)

A run of warm-up matmuls on a memset tile covers the initial DMA window
so the PE HAM clock-gate is released (2.4 GHz) before the real stream
begins. DMA issue order is arranged so enc blocks are never queued
behind small transfers.
"""

import numpy as np

B = 32
S = 2048
DD = 512
DE2 = 1024
NCORES = 8
BL = B // NCORES  # 4 batches per core
R = BL * S  # 8192 rows per core
NK = DE2 // 128  # 8 k-chunks
NO = DD // 128  # 4 o-chunks
NB2 = R // 1024  # 8 DMA blocks of 1024 rows
EXP_OFF = -20.0  # softmax shift; scores observed in [-32, 27]
NWARM = 88

_CACHE = {}


def _build_bass():
    import concourse.bacc as bacc
    import concourse.mybir as mybir
    import concourse.tile as tile
    from concourse._compat import get_trn_type

    f32 = mybir.dt.float32
    f16 = mybir.dt.float16
    AF = mybir.ActivationFunctionType

    nc = bacc.Bacc(get_trn_type() or "TRN2", target_bir_lowering=False, debug=False)

    encB = nc.dram_tensor("encB", [128, NB2 * NK * 1024], f16, kind="ExternalInput")
    enc_first = nc.dram_tensor("enc_first", [128, NK * 512], f16, kind="ExternalInput")
    w_eT = nc.dram_tensor("w_eT", [128, NO * NK * 128], f16, kind="ExternalInput")
    hb_in = nc.dram_tensor("hb_in", [128, NO * BL], f32, kind="ExternalInput")
    v_pb = nc.dram_tensor("v_pb", [128, NO], f32, kind="ExternalInput")
    v_pb16 = nc.dram_tensor("v_pb16", [128, NO], f16, kind="ExternalInput")
    probs = nc.dram_tensor("probs", [BL, S], f32, kind="ExternalOutput")

    with tile.TileContext(nc) as tc:
        with (
            tc.tile_pool(name="const", bufs=1) as const,
            tc.tile_pool(name="encp", bufs=6) as encp,
            tc.tile_pool(name="etp", bufs=8) as etp,
            tc.tile_pool(name="prp", bufs=10) as prp,
            tc.tile_pool(name="pep", bufs=4, space="PSUM") as pep,
            tc.tile_pool(name="pmisc", bufs=2, space="PSUM") as pmisc,
            tc.tile_pool(name="pwu", bufs=1, space="PSUM") as pwu,
        ):
            # ---- PE warm-up: dummy matmuls while DMAs stream in ----
            warm_sb = const.tile([128, 128], f16, name="warm_sb")
            nc.any.memset(warm_sb[:], 0.0)
            wu_ps = pwu.tile([128, 128], f32, name="wu_ps", tag="wu")
            for i in range(NWARM):
                nc.tensor.matmul(
                    wu_ps[:], warm_sb[:], warm_sb[:], start=True, stop=True
                )

            # ---- critical-path DMAs, dual-issue: enc stream on Sync,
            # ---- weights/bias channel on Scalar (also HWDGE) ----
            encB_v = encB[:].rearrange("p (t k r) -> p t k r", t=NB2, k=NK)
            enc_first_v = enc_first[:].rearrange("p (k r) -> p k r", k=NK)
            b0h0 = const.tile([128, NK, 512], f16, name="b0h0")
            nc.sync.dma_start(b0h0[:], enc_first_v[:])
            # we_sb[p, j, k, oo] = W_e[128j+oo, 128k+p]; per-j DMAs so the
            # first matmul group only waits on 256 KB of weights
            we_sb = const.tile([128, NO, NK, 128], f16, name="we_sb")
            we_v = w_eT[:].rearrange("p (j k oo) -> p j k oo", j=NO, k=NK)
            for j in range(NO):
                nc.scalar.dma_start(we_sb[:, j], we_v[:, j])
            hb_sb = const.tile([128, NO, BL], f32, name="hb_sb")
            nc.scalar.dma_start(hb_sb[:], hb_in[:].rearrange("p (j b) -> p j b", j=NO))
            v_sb = const.tile([128, NO], f32, name="v_sb")
            nc.scalar.dma_start(v_sb[:], v_pb[:])
            v16_sb = const.tile([128, NO], f16, name="v16_sb")
            nc.scalar.dma_start(v16_sb[:], v_pb16[:])
            b0h1 = const.tile([128, NK, 512], f16, name="b0h1")
            nc.sync.dma_start(b0h1[:], encB_v[:, 0, :, 512:1024])

            ones_v = const.tile([128, 1], f16, name="ones_v")
            nc.any.memset(ones_v[:], 1.0)
            expoff_sb = const.tile([1, 1], f32, name="expoff_sb")
            nc.any.memset(expoff_sb[:], EXP_OFF)
            expo_flat = const.tile([1, R], f32, name="expo_flat")
            sumparts = const.tile([1, 4 * BL], f32, name="sumparts")
            outp = const.tile([1, R], f32, name="outp")

            def emit_exp(sc, t_i):
                # streaming softmax numerator + partial sum
                nc.scalar.activation(
                    expo_flat[0:1, 512 * t_i : 512 * (t_i + 1)],
                    sc[:],
                    AF.Exp,
                    bias=expoff_sb[:],
                    accum_out=sumparts[0:1, t_i : t_i + 1],
                )

            def emit_finalize(b):
                rsum = const.tile([1, 1], f32, name=f"rsum{b}", tag=f"rs{b}")
                nc.vector.reduce_sum(
                    rsum[:],
                    sumparts[0:1, 4 * b : 4 * (b + 1)],
                    axis=mybir.AxisListType.X,
                )
                rec = const.tile([1, 1], f32, name=f"rec{b}", tag=f"rc{b}")
                nc.vector.reciprocal(rec[:], rsum[:])
                # halves: first store overlaps the second half's multiply
                for u in range(2):
                    lo2 = S * b + (S // 2) * u
                    hi2 = lo2 + S // 2
                    nc.vector.tensor_scalar_mul(
                        outp[0:1, lo2:hi2], expo_flat[0:1, lo2:hi2], rec[:]
                    )
                    nc.scalar.dma_start(
                        probs[b : b + 1, (S // 2) * u : (S // 2) * (u + 1)],
                        outp[0:1, lo2:hi2],
                    )

            def emit_score(st):
                # ones-matmul deferred one half-block: its DVE-tree input is
                # long finished, so the PE never stalls on sem 157
                p0, b0_, t0_ = st
                sc = pmisc.tile([1, 512], f32, name="sc", tag="mi")
                nc.tensor.matmul(sc[:], ones_v[:], p0[:], start=True, stop=True)
                emit_exp(sc, t0_)
                if t0_ % 4 == 3:
                    emit_finalize(b0_)

            # ---- main loop: 8 DMA blocks x 2 halves of 512 rows ----
            pending = []
            for t2 in range(NB2):
                # flush the previous block's two deferred scores as an
                # adjacent pair: one weight-switch in/out instead of two
                for st in pending:
                    emit_score(st)
                pending = []
                if t2 == 0:
                    halves = [b0h0, b0h1]
                else:
                    enc_t = encp.tile([128, NK, 1024], f16, name="enc_t", tag="enc")
                    nc.sync.dma_start(enc_t[:], encB_v[:, t2])
                    halves = [enc_t, enc_t]
                b = t2 // 2
                for h in range(2):
                    t_i = 2 * t2 + h  # 512-row block index, 4 per batch
                    last = t_i == 2 * NB2 - 1
                    src = halves[h]
                    lo = 0 if t2 == 0 else 512 * h
                    et_list = []
                    prods = []
                    for j in range(NO):
                        pe = pep.tile([128, 512], f32, name="pe", tag="pe")
                        for k in range(NK):
                            nc.tensor.matmul(
                                pe[:],
                                we_sb[:, j, k, :],
                                src[:, k, lo : lo + 512],
                                start=(k == 0),
                                stop=(k == NK - 1),
                            )
                        et = etp.tile([128, 512], f16, name="et", tag="et")
                        nc.scalar.activation(
                            et[:], pe[:], AF.Tanh, bias=hb_sb[:, j, b : b + 1]
                        )
                        et_list.append(et)
                        if not last:
                            # v-scale immediately so the DVE tree tracks the tanhs
                            pj = prp.tile([128, 512], f16, name=f"pj{j}", tag="pr")
                            nc.vector.tensor_scalar_mul(
                                pj[:], et[:], v_sb[:, j : j + 1]
                            )
                            prods.append(pj)
                    if last:
                        # final block: PE v-dot directly (shortest dep chain),
                        # after flushing this block's other deferred score
                        for st in pending:
                            emit_score(st)
                        pending = []
                        sc = pmisc.tile([1, 512], f32, name="sc", tag="mi")
                        for j in range(NO):
                            nc.tensor.matmul(
                                sc[:],
                                v16_sb[:, j : j + 1],
                                et_list[j][:],
                                start=(j == 0),
                                stop=(j == NO - 1),
                            )
                        emit_exp(sc, t_i)
                        emit_finalize(b)
                    else:
                        nc.vector.tensor_add(prods[0][:], prods[0][:], prods[1][:])
                        nc.vector.tensor_add(prods[2][:], prods[2][:], prods[3][:])
                        nc.vector.tensor_add(prods[0][:], prods[0][:], prods[2][:])
                        pending.append((prods[0], b, t_i))

    nc.compile()
    return nc


def _get_nc():
    if "nc" not in _CACHE:
        _CACHE["nc"] = _build_bass()
    return _CACHE["nc"]


def _tile_rows(mat_t, nchunk):
    # [nchunk*128, F] -> [128, nchunk*F] with out[p, c*F+f] = mat_t[128c+p, f]
    n, F = mat_t.shape
    assert n == nchunk * 128
    return np.ascontiguousarray(
        mat_t.reshape(nchunk, 128, F).transpose(1, 0, 2)
    ).reshape(128, nchunk * F)


def _make_in_maps(hidden, enc, W, b, v):
    W_h = W[:, :DD]
    W_e = W[:, DD:]
    # w_eT[p, j, k, oo] = W_e[128j+oo, 128k+p]
    w_eT = np.ascontiguousarray(
        W_e.reshape(NO, 128, NK, 128).transpose(3, 0, 2, 1)
    ).reshape(128, NO * NK * 128).astype(np.float16)
    v_pb = np.ascontiguousarray(v.reshape(NO, 128).T).astype(np.float32)
    v_pb16 = v_pb.astype(np.float16)
    enc16 = enc.astype(np.float16)  # [S, B, DE2]
    in_maps = []
    for c in range(NCORES):
        ec = enc16[:, BL * c : BL * (c + 1), :]  # [S, BL, DE2]
        encT = np.ascontiguousarray(ec.transpose(2, 1, 0)).reshape(DE2, R)
        # encB[p, t2, k, r] = encT[128k+p, 1024*t2 + r] (contiguous per block)
        encB = np.ascontiguousarray(
            encT.reshape(NK, 128, NB2, 1024).transpose(1, 2, 0, 3)
        ).reshape(128, NB2 * NK * 1024)
        enc_first = _tile_rows(np.ascontiguousarray(encT[:, :512]), NK)
        # exact f32 h-projection + bias, tiled per-partition: [128, (j, b)]
        h_proj = hidden[BL * c : BL * (c + 1), :] @ W_h.T + b  # [BL, DD]
        hb = _tile_rows(np.ascontiguousarray(h_proj.T), NO)  # [128, NO*BL]
        in_maps.append(
            {
                "encB": encB,
                "enc_first": enc_first,
                "w_eT": w_eT,
                "hb_in": np.ascontiguousarray(hb, dtype=np.float32),
                "v_pb": v_pb,
                "v_pb16": v_pb16,
            }
        )
    return in_maps


def kernel(hidden, encoder_outputs, W, b, v):
    """Full inputs in, full output out; 8-way batch-parallel inside."""
    from concourse.bass_utils import run_bass_kernel_spmd

    hidden = np.asarray(hidden, dtype=np.float32)
    enc = np.asarray(encoder_outputs, dtype=np.float32)
    W = np.asarray(W, dtype=np.float32)
    b = np.asarray(b, dtype=np.float32)
    v = np.asarray(v, dtype=np.float32)

    in_maps = _make_in_maps(hidden, enc, W, b, v)
    nc = _get_nc()
    res = run_bass_kernel_spmd(nc, in_maps, core_ids=list(range(NCORES)))
    out = np.concatenate([res.results[c]["probs"] for c in range(NCORES)], axis=0)
    return out.astype(np.float32)


# revision 46
# speedup vs baseline: 1.0344x; 1.0012x over previous
"""Bahdanau-attention scores kernel for Trainium2, 8-core data-parallel.

Computes softmax_s( v . tanh(W_h @ h[b] + W_e @ enc[s,b] + bias) ) for
B=32, S=2048, Dd=512, De2=1024, sharded 4 batches per NeuronCore.

Per-core device layout (host pre-shards / pre-tiles into per-partition
form so every DMA is 128 long contiguous runs; r = b_local*2048 + s):
  encB      [128, 8*8*1024] fp16 encB[p,(t,k,r)] = enc^T[128k+p, 1024t+r]
  enc_first [128, 8*512]  fp16  block-0 first half, pre-tiled contiguous
  w_eT      [128, 4*8*128] fp16 w_eT[p, (j,k,oo)] = W_e[128j+oo, 128k+p]
  hb_in     [128, 4*4]    f32   hb_in[p,(j,b)] = (hidden @ W_h^T + bias)[b, 128j+p]
  v_pb      [128, 4]      f32   v_pb[p, j] = v[128j + p]
  v_pb16    [128, 4]      fp16  same, fp16 (final-block PE v-dot)
Output:
  probs     [4, 2048]     f32

The h-projection (hidden @ W_h^T + bias, 0.02% of total FLOPs) is
precomputed on host in exact f32 and shipped as a per-partition bias
table; everything else runs on device:
  E^T[o, r]  = sum_k W_e^T[k, o] encT[k, r]            (PE fp16, 8 k-chunks)
  et[o, r]   = tanh(E^T + hb[:, b])                    (ACT, per-partition bias)
  prod[o, r] = et * v[o]  summed over 4 o-chunks       (DVE mul/add tree, fp16)
  sc[r]      = ones^T @ prod                           (PE, K=128 -> [1, 512])
  expo       = exp(sc - 20), partial sums via accum_out (ACT, streaming softmax)
  probs[b,:] = expo / sum(expo)                        (DVE, per-batch finalize)

A run of warm-up matmuls on a memset tile covers the initial DMA window
so the PE HAM clock-gate is released (2.4 GHz) before the real stream
begins. DMA issue order is arranged so enc blocks are never queued
behind small transfers.
"""

import numpy as np

B = 32
S = 2048
DD = 512
DE2 = 1024
NCORES = 8
BL = B // NCORES  # 4 batches per core
R = BL * S  # 8192 rows per core
NK = DE2 // 128  # 8 k-chunks
NO = DD // 128  # 4 o-chunks
NB2 = R // 1024  # 8 DMA blocks of 1024 rows
EXP_OFF = -20.0  # softmax shift; scores observed in [-32, 27]
NWARM = 88

_CACHE = {}


def _build_bass():
    import concourse.bacc as bacc
    import concourse.mybir as mybir
    import concourse.tile as tile
    from concourse._compat import get_trn_type

    f32 = mybir.dt.float32
    f16 = mybir.dt.float16
    AF = mybir.ActivationFunctionType

    nc = bacc.Bacc(get_trn_type() or "TRN2", target_bir_lowering=False, debug=False)

    encB = nc.dram_tensor("encB", [128, NB2 * NK * 1024], f16, kind="ExternalInput")
    enc_first = nc.dram_tensor("enc_first", [128, NK * 512], f16, kind="ExternalInput")
    w_eT = nc.dram_tensor("w_eT", [128, NO * NK * 128], f16, kind="ExternalInput")
    hb_in = nc.dram_tensor("hb_in", [128, NO * BL], f32, kind="ExternalInput")
    v_pb = nc.dram_tensor("v_pb", [128, NO], f32, kind="ExternalInput")
    v_pb16 = nc.dram_tensor("v_pb16", [128, NO], f16, kind="ExternalInput")
    probs = nc.dram_tensor("probs", [BL, S], f32, kind="ExternalOutput")

    with tile.TileContext(nc) as tc:
        with (
            tc.tile_pool(name="const", bufs=1) as const,
            tc.tile_pool(name="encp", bufs=6) as encp,
            tc.tile_pool(name="etp", bufs=8) as etp,
            tc.tile_pool(name="prp", bufs=12) as prp,
            tc.tile_pool(name="pep", bufs=4, space="PSUM") as pep,
            tc.tile_pool(name="pmisc", bufs=3, space="PSUM") as pmisc,
            tc.tile_pool(name="pwu", bufs=1, space="PSUM") as pwu,
        ):
            # ---- PE warm-up: dummy matmuls while DMAs stream in ----
            warm_sb = const.tile([128, 128], f16, name="warm_sb")
            nc.any.memset(warm_sb[:], 0.0)
            wu_ps = pwu.tile([128, 128], f32, name="wu_ps", tag="wu")
            for i in range(NWARM):
                nc.tensor.matmul(
                    wu_ps[:], warm_sb[:], warm_sb[:], start=True, stop=True
                )

            # ---- critical-path DMAs, dual-issue: enc stream on Sync,
            # ---- weights/bias channel on Scalar (also HWDGE) ----
            encB_v = encB[:].rearrange("p (t k r) -> p t k r", t=NB2, k=NK)
            enc_first_v = enc_first[:].rearrange("p (k r) -> p k r", k=NK)
            b0h0 = const.tile([128, NK, 512], f16, name="b0h0")
            nc.sync.dma_start(b0h0[:], enc_first_v[:])
            # we_sb[p, j, k, oo] = W_e[128j+oo, 128k+p]; per-j DMAs so the
            # first matmul group only waits on 256 KB of weights
            we_sb = const.tile([128, NO, NK, 128], f16, name="we_sb")
            we_v = w_eT[:].rearrange("p (j k oo) -> p j k oo", j=NO, k=NK)
            for j in range(NO):
                nc.scalar.dma_start(we_sb[:, j], we_v[:, j])
            hb_sb = const.tile([128, NO, BL], f32, name="hb_sb")
            nc.scalar.dma_start(hb_sb[:], hb_in[:].rearrange("p (j b) -> p j b", j=NO))
            v_sb = const.tile([128, NO], f32, name="v_sb")
            nc.scalar.dma_start(v_sb[:], v_pb[:])
            v16_sb = const.tile([128, NO], f16, name="v16_sb")
            nc.scalar.dma_start(v16_sb[:], v_pb16[:])
            b0h1 = const.tile([128, NK, 512], f16, name="b0h1")
            nc.sync.dma_start(b0h1[:], encB_v[:, 0, :, 512:1024])

            ones_v = const.tile([128, 1], f16, name="ones_v")
            nc.any.memset(ones_v[:], 1.0)
            expoff_sb = const.tile([1, 1], f32, name="expoff_sb")
            nc.any.memset(expoff_sb[:], EXP_OFF)
            expo_flat = const.tile([1, R], f32, name="expo_flat")
            sumparts = const.tile([1, 4 * BL], f32, name="sumparts")
            outp = const.tile([1, R], f32, name="outp")

            def emit_exp(sc, t_i):
                # streaming softmax numerator + partial sum
                nc.scalar.activation(
                    expo_flat[0:1, 512 * t_i : 512 * (t_i + 1)],
                    sc[:],
                    AF.Exp,
                    bias=expoff_sb[:],
                    accum_out=sumparts[0:1, t_i : t_i + 1],
                )

            def emit_finalize(b):
                rsum = const.tile([1, 1], f32, name=f"rsum{b}", tag=f"rs{b}")
                nc.vector.reduce_sum(
                    rsum[:],
                    sumparts[0:1, 4 * b : 4 * (b + 1)],
                    axis=mybir.AxisListType.X,
                )
                rec = const.tile([1, 1], f32, name=f"rec{b}", tag=f"rc{b}")
                nc.vector.reciprocal(rec[:], rsum[:])
                # halves: first store overlaps the second half's multiply
                for u in range(2):
                    lo2 = S * b + (S // 2) * u
                    hi2 = lo2 + S // 2
                    nc.vector.tensor_scalar_mul(
                        outp[0:1, lo2:hi2], expo_flat[0:1, lo2:hi2], rec[:]
                    )
                    nc.scalar.dma_start(
                        probs[b : b + 1, (S // 2) * u : (S // 2) * (u + 1)],
                        outp[0:1, lo2:hi2],
                    )

            def emit_score(st):
                # ones-matmul deferred one half-block: its DVE-tree input is
                # long finished, so the PE never stalls on sem 157
                p0, b0_, t0_ = st
                sc = pmisc.tile([1, 512], f32, name="sc", tag="mi")
                nc.tensor.matmul(sc[:], ones_v[:], p0[:], start=True, stop=True)
                emit_exp(sc, t0_)
                if t0_ % 4 == 3:
                    emit_finalize(b0_)

            # ---- main loop: 8 DMA blocks x 2 halves of 512 rows ----
            pending = []
            for t2 in range(NB2):
                if t2 == 0:
                    halves = [b0h0, b0h1]
                else:
                    enc_t = encp.tile([128, NK, 1024], f16, name="enc_t", tag="enc")
                    nc.sync.dma_start(enc_t[:], encB_v[:, t2])
                    halves = [enc_t, enc_t]
                b = t2 // 2
                for h in range(2):
                    t_i = 2 * t2 + h  # 512-row block index, 4 per batch
                    last = t_i == 2 * NB2 - 1
                    src = halves[h]
                    lo = 0 if t2 == 0 else 512 * h
                    if len(pending) >= 3:
                        # flush three deferred scores adjacently: one
                        # weight-switch in/out per trio instead of per matmul;
                        # every entry is at least a half-block old so the PE
                        # never waits on its DVE tree
                        for st in pending:
                            emit_score(st)
                        pending = []
                    et_list = []
                    prods = []
                    for j in range(NO):
                        pe = pep.tile([128, 512], f32, name="pe", tag="pe")
                        for k in range(NK):
                            nc.tensor.matmul(
                                pe[:],
                                we_sb[:, j, k, :],
                                src[:, k, lo : lo + 512],
                                start=(k == 0),
                                stop=(k == NK - 1),
                            )
                        et = etp.tile([128, 512], f16, name="et", tag="et")
                        nc.scalar.activation(
                            et[:], pe[:], AF.Tanh, bias=hb_sb[:, j, b : b + 1]
                        )
                        et_list.append(et)
                        if not last:
                            # v-scale immediately so the DVE tree tracks the tanhs
                            pj = prp.tile([128, 512], f16, name=f"pj{j}", tag="pr")
                            nc.vector.tensor_scalar_mul(
                                pj[:], et[:], v_sb[:, j : j + 1]
                            )
                            prods.append(pj)
                    if last:
                        # final block: PE v-dot directly (shortest dep chain),
                        # after flushing this block's other deferred score
                        for st in pending:
                            emit_score(st)
                        pending = []
                        sc = pmisc.tile([1, 512], f32, name="sc", tag="mi")
                        for j in range(NO):
                            nc.tensor.matmul(
                                sc[:],
                                v16_sb[:, j : j + 1],
                                et_list[j][:],
                                start=(j == 0),
                                stop=(j == NO - 1),
                            )
                        emit_exp(sc, t_i)
                        emit_finalize(b)
                    else:
                        nc.vector.tensor_add(prods[0][:], prods[0][:], prods[1][:])
                        nc.vector.tensor_add(prods[2][:], prods[2][:], prods[3][:])
                        nc.vector.tensor_add(prods[0][:], prods[0][:], prods[2][:])
                        pending.append((prods[0], b, t_i))

    nc.compile()
    return nc


def _get_nc():
    if "nc" not in _CACHE:
        _CACHE["nc"] = _build_bass()
    return _CACHE["nc"]


def _tile_rows(mat_t, nchunk):
    # [nchunk*128, F] -> [128, nchunk*F] with out[p, c*F+f] = mat_t[128c+p, f]
    n, F = mat_t.shape
    assert n == nchunk * 128
    return np.ascontiguousarray(
        mat_t.reshape(nchunk, 128, F).transpose(1, 0, 2)
    ).reshape(128, nchunk * F)


def _make_in_maps(hidden, enc, W, b, v):
    W_h = W[:, :DD]
    W_e = W[:, DD:]
    # w_eT[p, j, k, oo] = W_e[128j+oo, 128k+p]
    w_eT = np.ascontiguousarray(
        W_e.reshape(NO, 128, NK, 128).transpose(3, 0, 2, 1)
    ).reshape(128, NO * NK * 128).astype(np.float16)
    v_pb = np.ascontiguousarray(v.reshape(NO, 128).T).astype(np.float32)
    v_pb16 = v_pb.astype(np.float16)
    enc16 = enc.astype(np.float16)  # [S, B, DE2]
    in_maps = []
    for c in range(NCORES):
        ec = enc16[:, BL * c : BL * (c + 1), :]  # [S, BL, DE2]
        encT = np.ascontiguousarray(ec.transpose(2, 1, 0)).reshape(DE2, R)
        # encB[p, t2, k, r] = encT[128k+p, 1024*t2 + r] (contiguous per block)
        encB = np.ascontiguousarray(
            encT.reshape(NK, 128, NB2, 1024).transpose(1, 2, 0, 3)
        ).reshape(128, NB2 * NK * 1024)
        enc_first = _tile_rows(np.ascontiguousarray(encT[:, :512]), NK)
        # exact f32 h-projection + bias, tiled per-partition: [128, (j, b)]
        h_proj = hidden[BL * c : BL * (c + 1), :] @ W_h.T + b  # [BL, DD]
        hb = _tile_rows(np.ascontiguousarray(h_proj.T), NO)  # [128, NO*BL]
        in_maps.append(
            {
                "encB": encB,
                "enc_first": enc_first,
                "w_eT": w_eT,
                "hb_in": np.ascontiguousarray(hb, dtype=np.float32),
                "v_pb": v_pb,
                "v_pb16": v_pb16,
            }
        )
    return in_maps


def kernel(hidden, encoder_outputs, W, b, v):
    """Full inputs in, full output out; 8-way batch-parallel inside."""
    from concourse.bass_utils import run_bass_kernel_spmd

    hidden = np.asarray(hidden, dtype=np.float32)
    enc = np.asarray(encoder_outputs, dtype=np.float32)
    W = np.asarray(W, dtype=np.float32)
    b = np.asarray(b, dtype=np.float32)
    v = np.asarray(v, dtype=np.float32)

    in_maps = _make_in_maps(hidden, enc, W, b, v)
    nc = _get_nc()
    res = run_bass_kernel_spmd(nc, in_maps, core_ids=list(range(NCORES)))
    out = np.concatenate([res.results[c]["probs"] for c in range(NCORES)], axis=0)
    return out.astype(np.float32)
